# revision 1
# baseline (speedup 1.0000x reference)
"""GPT-NeoX attention (B=4, S=1024, D=2048, H=16) on 8 TRN2 NeuronCores.

Tensor-parallel over heads: 2 heads per core. Each core computes its slice
of the fused QKV projection, RoPE, causal attention, and writes the
transposed per-head output [hd, S]; the host concatenates heads.

All on-chip matmuls use float32r (full PE speed at free-dim>=256) with
fp32 PSUM accumulation. Layouts are chosen so no on-chip transposes are
needed:
  - x is fed transposed  xT[feature, token]
  - q,k are produced transposed  qT/kT[hd, token]  (RoPE applied in place)
  - v is produced natural  v[token, hd]  via a second projection pass
  - scores are computed transposed  sT[k_token, q_token]
  - out is produced transposed  oT[hd, q_token] = v.T @ expT
  - softmax sum over k = ones-vector matmul; normalization applied to oT
    via a K=1 broadcast matmul of the reciprocal row.
"""

import os

import numpy as np

import concourse.bass as bass
import concourse.tile as tile
from concourse import bacc, mybir

# Problem constants (contract: nn_GPTNeoXAttention, fixed shapes)
B, S, D = 4, 1024, 2048
H = 16
HD = 128  # head dim
NCORES = 8
HPC = H // NCORES  # heads per core
ROPE_BASE = 10000.0
T = B * S  # 4096 tokens
KC = D // 128  # 16 contraction chunks of the model dim
NSL = 512  # token-slice width for the qk projection
NHALF = S // NSL  # 2 slices per batch
QCH = S // 512  # q slices per sequence in attention
SCALE = 1.0 / float(np.sqrt(HD))

F32 = mybir.dt.float32
F32R = mybir.dt.float32r

_CACHE = {}


def _build_program():
    nc = bacc.Bacc(
        "TRN2", target_bir_lowering=False, debug=False, num_devices=NCORES
    )

    x_d = nc.dram_tensor("x", [128, KC, T], F32R, kind="ExternalInput")
    w_d = nc.dram_tensor("w", [128, KC, 6 * HD], F32R, kind="ExternalInput")
    bqk_d = nc.dram_tensor("bqk", [128, 4], F32, kind="ExternalInput")
    bv_d = nc.dram_tensor("bv", [128, 2 * HD], F32, kind="ExternalInput")
    cos_d = nc.dram_tensor("cosT", [128, S], F32, kind="ExternalInput")
    sin_d = nc.dram_tensor("sinS", [128, S], F32, kind="ExternalInput")
    mask_d = nc.dram_tensor("masks", [128, 4, 512], F32, kind="ExternalInput")
    rot_d = nc.dram_tensor("rotT", [128, 128], F32R, kind="ExternalInput")
    ones_d = nc.dram_tensor("ones", [128, 128], F32R, kind="ExternalInput")
    out_d = nc.dram_tensor("out", [HPC, HD, B, S], F32, kind="ExternalOutput")

    x_ap = x_d.ap()
    w_ap = w_d.ap()
    out_ap = out_d.ap()

    Exp = mybir.ActivationFunctionType.Exp
    Identity = mybir.ActivationFunctionType.Identity

    with tile.TileContext(nc) as tc:
        with (
            tc.tile_pool(name="singles", bufs=1) as singles,
            tc.tile_pool(name="xin", bufs=2) as xin_pool,
            tc.tile_pool(name="qk", bufs=6) as qk_pool,
            tc.tile_pool(name="vp", bufs=2) as v_pool,
            tc.tile_pool(name="expp", bufs=4) as exp_pool,
            tc.tile_pool(name="tmp", bufs=3) as tmp_pool,
            tc.tile_pool(name="outp", bufs=3) as out_pool,
            tc.tile_pool(name="rcp", bufs=2) as rcp_pool,
            tc.tile_pool(name="ps_mix", bufs=2, space="PSUM") as ps_mix,
            tc.tile_pool(name="ps_s", bufs=2, space="PSUM") as ps_s,
            tc.tile_pool(name="ps_o", bufs=2, space="PSUM") as ps_o,
            tc.tile_pool(name="ps_sum", bufs=2, space="PSUM") as ps_sum,
        ):
            w_sb = singles.tile([128, KC, 6 * HD], F32R)
            for kc in range(KC):
                nc.scalar.dma_start(out=w_sb[:, kc, :], in_=w_ap[:, kc, :])
            cos_sb = singles.tile([128, S], F32)
            nc.gpsimd.dma_start(out=cos_sb, in_=cos_d.ap())
            sin_sb = singles.tile([128, S], F32)
            nc.gpsimd.dma_start(out=sin_sb, in_=sin_d.ap())
            bqk_sb = singles.tile([128, 4], F32)
            nc.gpsimd.dma_start(out=bqk_sb, in_=bqk_d.ap())
            bv_sb = singles.tile([128, 2 * HD], F32)
            nc.gpsimd.dma_start(out=bv_sb, in_=bv_d.ap())
            mask_sb = singles.tile([128, 4, 512], F32)
            nc.gpsimd.dma_start(out=mask_sb, in_=mask_d.ap())
            rot_sb = singles.tile([128, 128], F32R)
            nc.gpsimd.dma_start(out=rot_sb, in_=rot_d.ap())
            # ones[128,128] lhsT: ones.T @ expT = sum over k, replicated
            # across all 128 output partitions (broadcast-ready layout)
            ones_sb = singles.tile([128, 128], F32R)
            nc.gpsimd.dma_start(out=ones_sb, in_=ones_d.ap())

            for b in range(B):
                # feature-major q/k tiles for this batch:
                # m=0: q head0, m=1: q head1, m=2: k head0, m=3: k head1
                qk_tiles = [
                    qk_pool.tile([128, S], F32R, tag="qkt", name=f"qkt_{b}_{i}")
                    for i in range(4)
                ]
                # natural-layout v for this batch: [token(128), chunk, 2*HD]
                v_sb = v_pool.tile([128, S // 128, 2 * HD], F32R)

                for half in range(NHALF):
                    t0 = b * S + half * NSL  # global token offset
                    xsb = xin_pool.tile([128, KC, NSL], F32R)
                    nc.sync.dma_start(out=xsb, in_=x_ap[:, :, t0 : t0 + NSL])

                    sl = slice(half * NSL, (half + 1) * NSL)
                    # ---- q/k projection (transposed out: [feature, token]) ----
                    for m in range(4):
                        ps = ps_mix.tile([128, NSL], F32, tag="ps")
                        for kc in range(KC):
                            nc.tensor.matmul(
                                ps,
                                w_sb[:, kc, m * 128 : (m + 1) * 128],
                                xsb[:, kc, :],
                                start=(kc == 0),
                                stop=(kc == KC - 1),
                            )
                        # bias add (per-partition scalar) on ACT, PSUM -> SBUF
                        qb = tmp_pool.tile([128, NSL], F32R, tag="qb")
                        nc.scalar.activation(
                            qb, ps, Identity, bias=bqk_sb[:, m : m + 1], scale=1.0
                        )
                        # RoPE: rotate_half via PE permutation matmul, then
                        # same-partition elementwise combine on DVE.
                        dst = qk_tiles[m][:, sl]
                        ps2 = ps_mix.tile([128, NSL], F32, tag="ps")
                        nc.tensor.matmul(
                            ps2,
                            rot_sb,
                            qb,
                            start=True,
                            stop=True,
                        )
                        tmp2 = tmp_pool.tile([128, NSL], F32, tag="tmp2")
                        nc.vector.tensor_mul(tmp2, ps2, sin_sb[:, sl])
                        nc.vector.tensor_mul(dst, qb, cos_sb[:, sl])
                        nc.vector.tensor_add(dst, dst, tmp2)

                    # ---- v projection (natural out: [token, feature]) ----
                    for t in range(NSL // 128):
                        psv = ps_mix.tile([128, 2 * HD], F32, tag="ps")
                        for kc in range(KC):
                            nc.tensor.matmul(
                                psv,
                                xsb[:, kc, t * 128 : (t + 1) * 128],
                                w_sb[:, kc, 4 * 128 : 6 * 128],
                                start=(kc == 0),
                                stop=(kc == KC - 1),
                            )
                        nc.vector.tensor_add(
                            v_sb[:, half * (NSL // 128) + t, :], psv, bv_sb
                        )

                # ---- attention for this batch ----
                for h in range(HPC):
                    qT = qk_tiles[h]
                    kT = qk_tiles[2 + h]
                    for qs in range(QCH):
                        nk = (qs * 512 + 512) // 128  # causal: k chunks needed
                        ps_out = ps_o.tile([128, 512], F32)
                        ps_sm = ps_sum.tile([128, 512], F32)
                        qsl = slice(qs * 512, (qs + 1) * 512)
                        for ki in range(nk):
                            pss = ps_s.tile([128, 512], F32, tag="sc")
                            nc.tensor.matmul(
                                pss,
                                kT[:, ki * 128 : (ki + 1) * 128],
                                qT[:, qsl],
                                start=True,
                                stop=True,
                            )
                            e = exp_pool.tile([128, 512], F32R, tag="e")
                            nc.scalar.activation(e, pss, Exp, scale=SCALE)
                            off = ki * 128 - qs * 512
                            if 0 <= off <= 384:
                                nc.vector.tensor_mul(
                                    e, e, mask_sb[:, off // 128, :]
                                )
                            nc.tensor.matmul(
                                ps_out,
                                v_sb[:, ki, h * HD : (h + 1) * HD],
                                e,
                                start=(ki == 0),
                                stop=(ki == nk - 1),
                            )
                            nc.tensor.matmul(
                                ps_sm,
                                ones_sb,
                                e,
                                start=(ki == 0),
                                stop=(ki == nk - 1),
                            )
                        rc = rcp_pool.tile([128, 512], F32)
                        nc.vector.reciprocal_approx_fast(out=rc, in_=ps_sm)
                        o = out_pool.tile([128, 512], F32)
                        nc.vector.tensor_mul(o, ps_out, rc)
                        nc.sync.dma_start(
                            out=out_ap[h, :, b, qsl], in_=o
                        )

    nc.compile()
    return nc


def _prep_shared(hidden_states):
    x2 = np.ascontiguousarray(hidden_states.reshape(T, D).T)  # [D, T]
    x_host = np.ascontiguousarray(
        x2.reshape(KC, 128, T).transpose(1, 0, 2)
    )  # [128, KC, T]

    inv = 1.0 / (ROPE_BASE ** (np.arange(0, HD, 2, dtype=np.float64) / HD))
    f = np.outer(inv, np.arange(S, dtype=np.float64))  # [64, S]
    cosT = np.concatenate([np.cos(f), np.cos(f)], axis=0).astype(np.float32)
    sinS = np.concatenate([np.sin(f), np.sin(f)], axis=0).astype(np.float32)

    p = np.arange(128)[:, None]
    fcol = np.arange(512)[None, :]
    masks = np.stack(
        [(fcol >= p + o).astype(np.float32) for o in (0, 128, 256, 384)], axis=1
    )  # [128, 4, 512]
    masks = np.ascontiguousarray(masks)

    # rotate_half as a matmul: out = lhsT.T @ rhs with lhsT = rotT gives
    # (R @ q)[i] = -q[i+64] (i<64), q[i-64] (i>=64)
    rotT = np.zeros((128, 128), np.float32)
    rotT[np.arange(64), np.arange(64) + 64] = 1.0
    rotT[np.arange(64) + 64, np.arange(64)] = -1.0
    return x_host, cosT, sinS, masks, rotT


def _core_rows(c):
    h0, h1 = 2 * c, 2 * c + 1
    rows = []
    for part in range(3):  # q, k, v blocks
        for h in (h0, h1):
            base = h * 3 * HD + part * HD
            rows.extend(range(base, base + HD))
    return np.asarray(rows)


def _prep_core(w_qkv, b_qkv, c):
    rows = _core_rows(c)
    wT = np.ascontiguousarray(w_qkv[rows, :].T)  # [D, 768]
    w_host = np.ascontiguousarray(
        wT.reshape(KC, 128, 6 * HD).transpose(1, 0, 2)
    )  # [128, KC, 768]
    b_sel = b_qkv[rows]
    bqk = np.ascontiguousarray(b_sel[: 4 * 128].reshape(4, 128).T)  # [128, 4]
    bv = np.ascontiguousarray(
        np.broadcast_to(b_sel[4 * 128 :], (128, 2 * HD))
    )  # [128, 256]
    return w_host, bqk, bv


def _make_in_maps(hidden_states, w_qkv, b_qkv):
    x_host, cosT, sinS, masks, rotT = _prep_shared(hidden_states)
    in_maps = []
    for c in range(NCORES):
        w_host, bqk, bv = _prep_core(w_qkv, b_qkv, c)
        in_maps.append(
            {
                "x": x_host,
                "w": w_host,
                "bqk": bqk,
                "bv": bv,
                "cosT": cosT,
                "sinS": sinS,
                "masks": masks,
                "rotT": rotT,
                "ones": np.ones((128, 128), np.float32),
            }
        )
    return in_maps


def _assemble(results):
    outs = np.stack([results[c]["out"] for c in range(NCORES)])
    # [NCORES, HPC, HD, B, S] -> [B, S, H*HD]
    return np.ascontiguousarray(
        outs.reshape(H, HD, B, S).transpose(2, 3, 0, 1).reshape(B, S, D)
    )


def run(hidden_states, w_qkv, b_qkv, trace=False):
    from concourse.bass_utils import run_bass_kernel_spmd

    if "nc" not in _CACHE:
        _CACHE["nc"] = _build_program()
    nc = _CACHE["nc"]
    in_maps = _make_in_maps(
        np.asarray(hidden_states, dtype=np.float32),
        np.asarray(w_qkv, dtype=np.float32),
        np.asarray(b_qkv, dtype=np.float32),
    )
    res = run_bass_kernel_spmd(
        nc, in_maps, core_ids=list(range(NCORES)), trace=trace
    )
    out = _assemble(res.results)
    return out, res


def kernel(hidden_states, w_qkv, b_qkv):
    trace = os.environ.get("KERNEL_TRACE", "0") == "1"
    out, _res = run(hidden_states, w_qkv, b_qkv, trace=trace)
    return out



# revision 5
# speedup vs baseline: 1.3470x; 1.3470x over previous
"""GPT-NeoX attention (B=4, S=1024, D=2048, H=16) on 8 TRN2 NeuronCores.

Tensor-parallel over heads: 2 heads per core. Each core computes its slice
of the fused QKV projection, RoPE, causal attention, and writes the
transposed per-head output [hd, S]; the host concatenates heads.

v2 design:
  - bf16 matmul operands everywhere (x, w, q/k, v, e); fp32 PSUM. Validated
    numerically: rel err ~5e-3 vs the 2e-2 gate.
  - Startup streaming: m-major weight DMAs + 4-chunk x DMAs so the first
    projection chain starts ~8us in; a warm-up matmul chain keeps the PE
    clock ramp going until real data lands.
  - Attention software pipeline: scores matmuls issue LOOKAHEAD blocks
    ahead of the PV/sum matmuls so the exp (ACT) latency never stalls PE.
  - Causal column restriction: diagonal blocks only compute score/exp/PV/sum
    columns [off:512); fully-masked columns are zeroed via cheap Pool
    memsets of e; the triangular window is masked with one shared [128,128]
    lower-triangle multiply on DVE.
  - RoPE rotate-half via SBUF->SBUF DMA partition swap (sign folded into
    the sin table) instead of PE permutation matmuls.
"""

import os

import numpy as np
import ml_dtypes

import concourse.bass as bass
import concourse.tile as tile
from concourse import bacc, mybir

# Problem constants (contract: nn_GPTNeoXAttention, fixed shapes)
B, S, D = 4, 1024, 2048
H = 16
HD = 128  # head dim
NCORES = 8
HPC = H // NCORES  # heads per core
ROPE_BASE = 10000.0
T = B * S  # 4096 tokens
KC = D // 128  # 16 contraction chunks of the model dim
NSL = 512  # token-slice width per projection pass
NHALF = S // NSL  # 2 slices per batch
QCH = S // 512  # q slices per sequence in attention
SCALE = 1.0 / float(np.sqrt(HD))
LOOKAHEAD = 4  # scores blocks in flight ahead of PV consumption
NWARM = 22  # warm-up matmuls bridging the startup DMA window

F32 = mybir.dt.float32
F32R = mybir.dt.float32r
BF16 = mybir.dt.bfloat16
NP_BF16 = ml_dtypes.bfloat16

_CACHE = {}


def _build_program():
    nc = bacc.Bacc(
        "TRN2", target_bir_lowering=False, debug=False, num_devices=NCORES
    )

    # x8[p, bh, kc, s] = x[kc*128+p, bh*512+s]  (feature-major tokens)
    x_d = nc.dram_tensor("x8", [128, B * NHALF, KC, NSL], BF16,
                         kind="ExternalInput")
    # wqk[p, m, kc, f]: m in (q_h0, q_h1, k_h0, k_h1); lhsT chunks
    wqk_d = nc.dram_tensor("wqk", [128, 4, KC, 128], BF16,
                           kind="ExternalInput")
    # wv[p, kc, f]: rhs for the natural-layout v projection (2 heads x 128)
    wv_d = nc.dram_tensor("wv", [128, KC, 2 * HD], BF16,
                          kind="ExternalInput")
    bqk_d = nc.dram_tensor("bqk", [128, 4], F32, kind="ExternalInput")
    bv_d = nc.dram_tensor("bv", [128, 2 * HD], F32, kind="ExternalInput")
    cos_d = nc.dram_tensor("cosT", [128, S], BF16, kind="ExternalInput")
    # sinF is sign-folded: row i holds -sin for i<64, +sin for i>=64
    sin_d = nc.dram_tensor("sinF", [128, S], BF16, kind="ExternalInput")
    # tri[p, j] = 1 if j >= p else 0 (keep) -- shared diagonal-window mask
    tri_d = nc.dram_tensor("tri", [128, 128], BF16, kind="ExternalInput")
    ones_d = nc.dram_tensor("ones", [128, 128], BF16, kind="ExternalInput")
    out_d = nc.dram_tensor("out", [HPC, HD, B, S], F32, kind="ExternalOutput")

    x_ap = x_d.ap()
    out_ap = out_d.ap()

    Exp = mybir.ActivationFunctionType.Exp
    Identity = mybir.ActivationFunctionType.Identity

    with tile.TileContext(nc) as tc:
        with (
            tc.tile_pool(name="singles", bufs=1) as singles,
            tc.tile_pool(name="xin", bufs=3) as xin_pool,
            tc.tile_pool(name="qk", bufs=8) as qk_pool,
            tc.tile_pool(name="vp", bufs=2) as v_pool,
            tc.tile_pool(name="qb", bufs=3) as qb_pool,
            tc.tile_pool(name="swp", bufs=3) as swp_pool,
            tc.tile_pool(name="expp", bufs=8) as exp_pool,
            tc.tile_pool(name="outp", bufs=3) as out_pool,
            tc.tile_pool(name="rcp", bufs=2) as rcp_pool,
            tc.tile_pool(name="warmp", bufs=1) as warm_pool,
            tc.tile_pool(name="pp", bufs=4, space="PSUM") as pp,
            tc.tile_pool(name="po", bufs=2, space="PSUM") as po,
            tc.tile_pool(name="psm", bufs=2, space="PSUM") as psm,
        ):
            # ---- input DMAs (priority order: first x half, then weights) --
            wqk_sb = singles.tile([128, 4, KC, 128], BF16)
            wv_sb = singles.tile([128, KC, 2 * HD], BF16)
            bqk_sb = singles.tile([128, 4], F32)
            bv_sb = singles.tile([128, 2 * HD], F32)
            cos_sb = singles.tile([128, S], BF16)
            sin_sb = singles.tile([128, S], BF16)
            tri_sb = singles.tile([128, 128], BF16)
            ones_sb = singles.tile([128, 128], BF16)

            # ---- PE warm-up: hold the clock ramp while DMA streams in ----
            warm_sb = warm_pool.tile([128, 512], BF16)
            nc.gpsimd.memset(warm_sb, 0.0)
            ps_warm = pp.tile([128, 512], F32, tag="ps")
            for _ in range(NWARM):
                nc.tensor.matmul(
                    ps_warm, warm_sb[:, :128], warm_sb, start=True, stop=True
                )

            # x(b0,h0) chunks first on the sync hwdge queue; the big weight
            # DMAs on the scalar hwdge queue; small constants on gpsimd
            # (software DGE). Keeps the startup-critical transfers unblocked.
            xsb0 = xin_pool.tile([128, KC, NSL], BF16, tag="x")
            for c in range(4):
                nc.sync.dma_start(
                    out=xsb0[:, 4 * c : 4 * c + 4, :],
                    in_=x_ap[:, 0, 4 * c : 4 * c + 4, :],
                )
            for m in range(4):
                nc.scalar.dma_start(
                    out=wqk_sb[:, m], in_=wqk_d.ap()[:, m]
                )
            nc.scalar.dma_start(out=wv_sb, in_=wv_d.ap())
            nc.gpsimd.dma_start(out=bqk_sb, in_=bqk_d.ap())
            nc.gpsimd.dma_start(out=bv_sb, in_=bv_d.ap())
            nc.gpsimd.dma_start(out=cos_sb, in_=cos_d.ap())
            nc.gpsimd.dma_start(out=sin_sb, in_=sin_d.ap())
            nc.gpsimd.dma_start(out=tri_sb, in_=tri_d.ap())
            nc.gpsimd.dma_start(out=ones_sb, in_=ones_d.ap())
            # drain the warm-up psum (GPSIMD cannot access PSUM; DVE is idle
            # at startup, and its first real op comes well after this)
            warm_out = warm_pool.tile([128, 1], F32)
            nc.vector.tensor_copy(warm_out, ps_warm[:, 0:1])

            for b in range(B):
                # feature-major ROTATED q/k tiles for this batch:
                # m=0: q head0, m=1: q head1, m=2: k head0, m=3: k head1
                qk_tiles = [
                    qk_pool.tile([128, S], BF16, tag="qkt", name=f"qkt_{b}_{i}")
                    for i in range(4)
                ]
                # natural-layout v for this batch: [token(128), chunk, 2*HD]
                v_sb = v_pool.tile([128, S // 128, 2 * HD], BF16)

                for half in range(NHALF):
                    bh = b * NHALF + half
                    if bh == 0:
                        xsb = xsb0
                    else:
                        xsb = xin_pool.tile([128, KC, NSL], BF16, tag="x")
                        for c in range(4):
                            nc.sync.dma_start(
                                out=xsb[:, 4 * c : 4 * c + 4, :],
                                in_=x_ap[:, bh, 4 * c : 4 * c + 4, :],
                            )

                    sl = slice(half * NSL, (half + 1) * NSL)
                    # ---- q/k projection (transposed out: [feature, token])
                    for m in range(4):
                        ps = pp.tile([128, NSL], F32, tag="ps")
                        for kc in range(KC):
                            nc.tensor.matmul(
                                ps,
                                wqk_sb[:, m, kc, :],
                                xsb[:, kc, :],
                                start=(kc == 0),
                                stop=(kc == KC - 1),
                            )
                        # bias add (per-partition scalar) on ACT, PSUM->SBUF
                        qb = qb_pool.tile([128, NSL], BF16, tag="qb")
                        nc.scalar.activation(
                            qb, ps, Identity, bias=bqk_sb[:, m : m + 1],
                            scale=1.0,
                        )
                        # rotate-half via DMA partition swap (sign folded
                        # into sinF), then elementwise combine on DVE.
                        swp = swp_pool.tile([128, NSL], BF16, tag="sw")
                        nc.sync.dma_start(out=swp[0:64, :], in_=qb[64:128, :])
                        nc.sync.dma_start(out=swp[64:128, :], in_=qb[0:64, :])
                        dst = qk_tiles[m][:, sl]
                        nc.vector.tensor_mul(dst, qb, cos_sb[:, sl])
                        nc.vector.tensor_mul(swp, swp, sin_sb[:, sl])
                        nc.vector.tensor_add(dst, dst, swp)

                    # ---- v projection (natural out: [token, feature]) ----
                    for t in range(NSL // 128):
                        psv = pp.tile([128, NSL], F32, tag="ps")
                        for kc in range(KC):
                            nc.tensor.matmul(
                                psv[:, 0 : 2 * HD],
                                xsb[:, kc, t * 128 : (t + 1) * 128],
                                wv_sb[:, kc, :],
                                start=(kc == 0),
                                stop=(kc == KC - 1),
                            )
                        nc.vector.tensor_add(
                            v_sb[:, half * (NSL // 128) + t, :],
                            psv[:, 0 : 2 * HD],
                            bv_sb,
                        )

                # ---- attention for this batch ----
                for h in range(HPC):
                    qT = qk_tiles[h]
                    kT = qk_tiles[2 + h]
                    for qs in range(QCH):
                        nk = (qs * 512 + 512) // 128  # causal k chunks
                        ps_out = po.tile([128, 512], F32)
                        ps_sm = psm.tile([128, 512], F32)
                        qsl0 = qs * 512
                        e_tiles = [None] * nk

                        def emit_scores(ki):
                            off = ki * 128 - qs * 512  # >=0 on diagonal
                            lo = max(off, 0)
                            pss = pp.tile([128, 512], F32, tag="ps")
                            nc.tensor.matmul(
                                pss[:, lo:512],
                                kT[:, ki * 128 : (ki + 1) * 128],
                                qT[:, qsl0 + lo : qsl0 + 512],
                                start=True,
                                stop=True,
                            )
                            e = exp_pool.tile([128, 512], BF16, tag="e")
                            if lo > 0:
                                nc.gpsimd.memset(e[:, 0:lo], 0.0)
                            nc.scalar.activation(
                                e[:, lo:512], pss[:, lo:512], Exp, scale=SCALE
                            )
                            if off >= 0:
                                nc.vector.tensor_mul(
                                    e[:, off : off + 128],
                                    e[:, off : off + 128],
                                    tri_sb,
                                )
                            e_tiles[ki] = e

                        def emit_pv(ki):
                            off = ki * 128 - qs * 512
                            lo = max(off, 0)
                            e = e_tiles[ki]
                            nc.tensor.matmul(
                                ps_out[:, lo:512],
                                v_sb[:, ki, h * HD : (h + 1) * HD],
                                e[:, lo:512],
                                start=(ki == 0),
                                stop=(ki == nk - 1),
                            )
                            nc.tensor.matmul(
                                ps_sm[:, lo:512],
                                ones_sb,
                                e[:, lo:512],
                                start=(ki == 0),
                                stop=(ki == nk - 1),
                            )

                        for ki in range(min(LOOKAHEAD, nk)):
                            emit_scores(ki)
                        for ki in range(nk):
                            emit_pv(ki)
                            if ki + LOOKAHEAD < nk:
                                emit_scores(ki + LOOKAHEAD)

                        rc = rcp_pool.tile([128, 512], F32)
                        nc.vector.reciprocal_approx_fast(out=rc, in_=ps_sm)
                        o = out_pool.tile([128, 512], F32)
                        nc.vector.tensor_mul(o, ps_out, rc)
                        nc.scalar.dma_start(
                            out=out_ap[h, :, b, qsl0 : qsl0 + 512], in_=o
                        )

    nc.compile()
    return nc


def _prep_shared(hidden_states):
    x2 = np.ascontiguousarray(hidden_states.reshape(T, D).T)  # [D, T]
    # x8[p, bh, kc, s] = x2[kc*128+p, bh*512+s]
    x8 = np.ascontiguousarray(
        x2.reshape(KC, 128, B * NHALF, NSL).transpose(1, 2, 0, 3)
    ).astype(NP_BF16)

    inv = 1.0 / (ROPE_BASE ** (np.arange(0, HD, 2, dtype=np.float64) / HD))
    f = np.outer(inv, np.arange(S, dtype=np.float64))  # [64, S]
    cosT = np.concatenate([np.cos(f), np.cos(f)], axis=0).astype(NP_BF16)
    sinF = np.concatenate([-np.sin(f), np.sin(f)], axis=0).astype(NP_BF16)

    p = np.arange(128)[:, None]
    j = np.arange(128)[None, :]
    tri = (j >= p).astype(NP_BF16)  # keep j >= p in the diagonal window
    ones = np.ones((128, 128), NP_BF16)
    return x8, cosT, sinF, tri, ones


def _core_rows(c):
    h0, h1 = 2 * c, 2 * c + 1
    rows = []
    for part in range(3):  # q, k, v blocks
        for h in (h0, h1):
            base = h * 3 * HD + part * HD
            rows.extend(range(base, base + HD))
    return np.asarray(rows)


def _prep_core(w_qkv, b_qkv, c):
    rows = _core_rows(c)
    wsel = w_qkv[rows, :]  # [768, D]; order: q0,q1,k0,k1,v0,v1
    # wqk[p, m, kc, f] = wsel[m*128+f, kc*128+p]
    wqk = np.ascontiguousarray(
        wsel[: 4 * 128, :].reshape(4, 128, KC, 128).transpose(3, 0, 2, 1)
    ).astype(NP_BF16)
    # wv[p, kc, f] = wsel[512+f, kc*128+p]
    wv = np.ascontiguousarray(
        wsel[4 * 128 :, :].reshape(2 * HD, KC, 128).transpose(2, 1, 0)
    ).astype(NP_BF16)
    b_sel = b_qkv[rows]
    bqk = np.ascontiguousarray(
        b_sel[: 4 * 128].reshape(4, 128).T
    ).astype(np.float32)  # [128, 4]
    bv = np.ascontiguousarray(
        np.broadcast_to(b_sel[4 * 128 :], (128, 2 * HD))
    ).astype(np.float32)  # [128, 256]
    return wqk, wv, bqk, bv


def _make_in_maps(hidden_states, w_qkv, b_qkv):
    x8, cosT, sinF, tri, ones = _prep_shared(hidden_states)
    in_maps = []
    for c in range(NCORES):
        wqk, wv, bqk, bv = _prep_core(w_qkv, b_qkv, c)
        in_maps.append(
            {
                "x8": x8,
                "wqk": wqk,
                "wv": wv,
                "bqk": bqk,
                "bv": bv,
                "cosT": cosT,
                "sinF": sinF,
                "tri": tri,
                "ones": ones,
            }
        )
    return in_maps


def _assemble(results):
    outs = np.stack([results[c]["out"] for c in range(NCORES)])
    # [NCORES, HPC, HD, B, S] -> [B, S, H*HD]
    return np.ascontiguousarray(
        outs.reshape(H, HD, B, S).transpose(2, 3, 0, 1).reshape(B, S, D)
    )


def run(hidden_states, w_qkv, b_qkv, trace=False):
    from concourse.bass_utils import run_bass_kernel_spmd

    if "nc" not in _CACHE:
        _CACHE["nc"] = _build_program()
    nc = _CACHE["nc"]
    in_maps = _make_in_maps(
        np.asarray(hidden_states, dtype=np.float32),
        np.asarray(w_qkv, dtype=np.float32),
        np.asarray(b_qkv, dtype=np.float32),
    )
    res = run_bass_kernel_spmd(
        nc, in_maps, core_ids=list(range(NCORES)), trace=trace
    )
    out = _assemble(res.results)
    return out, res


def kernel(hidden_states, w_qkv, b_qkv):
    trace = os.environ.get("KERNEL_TRACE", "0") == "1"
    out, _res = run(hidden_states, w_qkv, b_qkv, trace=trace)
    return out
